# revision 12
# baseline (speedup 1.0000x reference)
"""Trainium2 Bass kernel for nn_EstLossSepEmb (contrastive eval loss_fn).

Strategy (data-parallel over the batch dim, 8 cores, 1024 rows each):
  - Host prep (layout only): slice + transpose each core's query-side tensors
    to [256, 1024]; ship the full caption_emb transposed and *rolled* so each
    core's own 1024 text rows come first (gives compile-time diagonal offsets
    in an SPMD kernel).
  - Device:
      * normalize caption_emb columns (t-side): square (ACT) -> column-sums
        via ones-matmul (PE) -> sqrt (ACT) -> reciprocal (DVE) -> broadcast
        (GPSIMD) -> scale in place (GPSIMD).
      * query side (v / gt_v / narr_v) is NOT normalized: per-row argmax is
        scale-invariant in the row, so only the t-side norms matter.
      * 3 big sim matmuls [1024,256]x[256,8192] in fp32r (FP22 multiply,
        1 cycle/row, error ~1e-5 << observed decision margins ~3e-2).
      * diagonal d_i = <x_i, t_i> via elementwise product + ones-matmul.
      * per row decide "argmax == i" as (d+TOL >= rowmax) over DVE-reduced
        n-chunks AND count(sim > d+TOL)==0 over ACT Sign+accum n-chunks.
      * rowwise-cos loss ingredients (dots + sumsqs) via ones-matmuls.
  - Host combine: means / cos / counts -> the 9-vector output.
"""

import os

import numpy as np

BB = 8192
DIM = 256
NCORES = 8
RPC = BB // NCORES  # rows per core = 1024
MB = RPC // 128  # m blocks per core = 8
NTILE = 512
NCH = BB // NTILE  # 16 n-chunks
KCH = DIM // 128  # 2 k-chunks
TOL = 1e-4
EPS = 1e-8
N_ACT = int(os.environ.get("K_NACT", "8"))  # n-chunks counted on ACT
N_DVE = NCH - N_ACT  # n-chunks max-reduced on DVE

Q_NAMES = ["q_dot_vf", "q_ss_v", "q_ss_gv", "q_dot_tc", "q_ss_tp", "q_ss_ce"]
X_NAMES = ["v", "gv", "nv"]

_built = None


def _build_nc():
    import concourse.bacc as bacc
    import concourse.mybir as mybir
    import concourse.tile as tile

    F32 = mybir.dt.float32
    F32R = mybir.dt.float32r
    AF = mybir.ActivationFunctionType
    AX = mybir.AxisListType

    nc = bacc.Bacc("TRN2", target_bir_lowering=False, debug=False)

    # Tensors feeding fp32r matmuls must be float32r end-to-end (the BIR
    # verifier requires producers to write FP22-rounded values).
    d_in = {}
    for nm in ["vt", "gvt", "nvt"]:
        d_in[nm] = nc.dram_tensor(nm, [DIM, RPC], F32R, kind="ExternalInput")
    for nm in ["tpt", "cet"]:
        d_in[nm] = nc.dram_tensor(nm, [DIM, RPC], F32, kind="ExternalInput")
    d_in["cer"] = nc.dram_tensor("cer", [DIM, BB], F32R, kind="ExternalInput")

    d_out = {}
    for nm in Q_NAMES:
        d_out[nm] = nc.dram_tensor(nm, [128, MB], F32, kind="ExternalOutput")
    for x in X_NAMES:
        for pre in ["dplus_", "mx_", "cnt_"]:
            nm = pre + x
            d_out[nm] = nc.dram_tensor(nm, [128, MB], F32, kind="ExternalOutput")
    d_out["sdbg"] = nc.dram_tensor("sdbg", [1, BB], F32, kind="ExternalOutput")



    with tile.TileContext(nc) as tc:
        with (
            tc.tile_pool(name="per", bufs=1) as per,
            tc.tile_pool(name="sc", bufs=3) as sc,
            tc.tile_pool(name="pr", bufs=2) as pr,
            tc.tile_pool(name="psb", bufs=4, space="PSUM") as psb,
            tc.tile_pool(name="pss", bufs=2, space="PSUM") as pss,
        ):
            # ---- persistent SBUF tiles ----
            ce = [per.tile([128, BB], F32R, name=f"ce{k}") for k in range(KCH)]
            xin = {}
            for nm in ["vt", "gvt", "nvt"]:
                xin[nm] = [
                    per.tile([128, RPC], F32R, name=f"{nm}{k}") for k in range(KCH)
                ]
            for nm in ["tpt", "cet"]:
                xin[nm] = [
                    per.tile([128, RPC], F32, name=f"{nm}{k}") for k in range(KCH)
                ]
            s = per.tile([1, BB], F32, name="s")
            # N=2 ones: fp32 matmul outputs must be 8-byte (2-elem) PSUM lines
            ones_f = per.tile([128, 2], F32, name="ones_f")
            nc.gpsimd.memset(ones_f[:], 1.0)
            ones = per.tile([128, 2], F32R, name="ones")
            nc.vector.tensor_copy(ones[:], ones_f[:])

            negd = {}
            dplus = {}
            mxsl = {}
            cntsl = {}
            mxr = {}
            cntr = {}
            for x in X_NAMES:
                negd[x] = per.tile([128, MB], F32, name=f"negd_{x}")
                dplus[x] = per.tile([128, MB], F32, name=f"dplus_{x}")
                mxsl[x] = per.tile([128, MB, N_DVE], F32, name=f"mxsl_{x}")
                cntsl[x] = per.tile([128, MB, N_ACT], F32, name=f"cntsl_{x}")
                mxr[x] = per.tile([128, MB], F32, name=f"mxr_{x}")
                cntr[x] = per.tile([128, MB], F32, name=f"cntr_{x}")

            # ---- input DMAs ----
            for nm in ["vt", "gvt", "nvt", "tpt", "cet"]:
                for k in range(KCH):
                    nc.sync.dma_start(
                        xin[nm][k][:], d_in[nm][k * 128 : (k + 1) * 128, :]
                    )
            for k in range(KCH):
                for n in range(NCH):
                    cs = slice(n * NTILE, (n + 1) * NTILE)
                    nc.sync.dma_start(
                        ce[k][:, cs], d_in["cer"][k * 128 : (k + 1) * 128, cs]
                    )

            # ---- t-side column norms: s = 1/sqrt(colsum(ce^2)) ----
            for n in range(NCH):
                cs = slice(n * NTILE, (n + 1) * NTILE)
                psn = pss.tile([1, NTILE], F32, name="psn")
                for k in range(KCH):
                    sq = sc.tile([128, NTILE], F32R, name="sq")
                    nc.scalar.activation(sq[:], ce[k][:, cs], AF.Square)
                    nc.tensor.matmul(
                        psn[:],
                        ones[:, 0:1],
                        sq[:],
                        start=(k == 0),
                        stop=(k == KCH - 1),
                    )
                nc.scalar.activation(s[0:1, cs], psn[:], AF.Sqrt)
                nc.vector.reciprocal(s[0:1, cs], s[0:1, cs])

            nc.sync.dma_start(d_out["sdbg"][:], s[:])

            # ---- scale ce columns in place (GPSIMD) ----
            for n in range(NCH):
                cs = slice(n * NTILE, (n + 1) * NTILE)
                bc = sc.tile([128, NTILE], F32, name="bc")
                nc.gpsimd.partition_broadcast(bc[:], s[0:1, cs])
                for k in range(KCH):
                    nc.gpsimd.tensor_mul(ce[k][:, cs], ce[k][:, cs], bc[:])

            # ---- diagonal d_i = <x_i, gt_t_i> (+TOL), per query tensor ----
            for x, nm in zip(X_NAMES, ["vt", "gvt", "nvt"]):
                prods = []
                for k in range(KCH):
                    prod = pr.tile([128, RPC], F32R, name=f"prod{k}")
                    nc.gpsimd.tensor_mul(prod[:], xin[nm][k][:], ce[k][:, 0:RPC])
                    prods.append(prod)
                psq = pss.tile([128, MB, 2], F32, name="psq")
                for m in range(MB):
                    ms = slice(m * 128, (m + 1) * 128)
                    for k in range(KCH):
                        nc.tensor.matmul(
                            psq[:, m, :],
                            prods[k][:, ms],
                            ones[:],
                            start=(k == 0),
                            stop=(k == KCH - 1),
                        )
                nc.vector.tensor_scalar_add(dplus[x][:], psq[:, :, 0:1], TOL)
                nc.vector.tensor_scalar_mul(negd[x][:], dplus[x][:], -1.0)
                nc.sync.dma_start(d_out["dplus_" + x][:], dplus[x][:])

            # ---- rowwise-cos loss ingredients ----
            def colsum_out(name, make_elem):
                psq = pss.tile([128, MB, 2], F32, name="psq")
                elems = [make_elem(k) for k in range(KCH)]
                for m in range(MB):
                    ms = slice(m * 128, (m + 1) * 128)
                    for k in range(KCH):
                        nc.tensor.matmul(
                            psq[:, m, :],
                            elems[k][:, ms],
                            ones[:],
                            start=(k == 0),
                            stop=(k == KCH - 1),
                        )
                qsb = sc.tile([128, MB], F32, name="qsb")
                nc.scalar.copy(qsb[:], psq[:, :, 0:1])
                nc.sync.dma_start(d_out[name][:], qsb[:])

            def mk_mul(a, b):
                def f(k):
                    prod = pr.tile([128, RPC], F32R, name=f"prod{k}")
                    nc.gpsimd.tensor_mul(prod[:], xin[a][k][:], xin[b][k][:])
                    return prod

                return f

            def mk_sq(a):
                def f(k):
                    prod = pr.tile([128, RPC], F32R, name=f"prod{k}")
                    nc.scalar.activation(prod[:], xin[a][k][:], AF.Square)
                    return prod

                return f

            colsum_out("q_dot_vf", mk_mul("vt", "gvt"))
            colsum_out("q_ss_v", mk_sq("vt"))
            colsum_out("q_ss_gv", mk_sq("gvt"))
            colsum_out("q_dot_tc", mk_mul("tpt", "cet"))
            colsum_out("q_ss_tp", mk_sq("tpt"))
            colsum_out("q_ss_ce", mk_sq("cet"))

            # ---- big sim matmuls + row max / count consumers ----
            xtiles = {"v": xin["vt"], "gv": xin["gvt"], "nv": xin["nvt"]}
            for n in range(NCH):
                cs = slice(n * NTILE, (n + 1) * NTILE)
                for x in X_NAMES:
                    xt = xtiles[x]
                    for m in range(MB):
                        ms = slice(m * 128, (m + 1) * 128)
                        pb = psb.tile([128, NTILE], F32, name="pb")
                        for k in range(KCH):
                            nc.tensor.matmul(
                                pb[:],
                                xt[k][:, ms],
                                ce[k][:, cs],
                                start=(k == 0),
                                stop=(k == KCH - 1),
                            )
                        if n < N_ACT:
                            dump = sc.tile([128, NTILE], F32, name="dump")
                            nc.scalar.activation(
                                dump[:],
                                pb[:],
                                AF.Sign,
                                bias=negd[x][:, m : m + 1],
                                accum_out=cntsl[x][:, m, n : n + 1],
                            )
                        else:
                            nc.vector.reduce_max(
                                mxsl[x][:, m, n - N_ACT : n - N_ACT + 1],
                                pb[:],
                                axis=AX.X,
                            )

            # ---- fold slots, write outputs ----
            for x in X_NAMES:
                nc.vector.reduce_max(mxr[x][:], mxsl[x][:], axis=AX.X)
                nc.vector.reduce_sum(cntr[x][:], cntsl[x][:], axis=AX.X)
                nc.sync.dma_start(d_out["mx_" + x][:], mxr[x][:])
                nc.sync.dma_start(d_out["cnt_" + x][:], cntr[x][:])

    nc.compile()
    return nc


def _get_nc():
    global _built
    if _built is None:
        _built = _build_nc()
    return _built


def _make_in_maps(inputs):
    vp = np.ascontiguousarray(np.asarray(inputs["vis_pred"], dtype=np.float32))
    tp = np.ascontiguousarray(np.asarray(inputs["text_pred"], dtype=np.float32))
    gv = np.ascontiguousarray(np.asarray(inputs["vis_feats_proj"], dtype=np.float32))
    ce = np.ascontiguousarray(np.asarray(inputs["caption_emb"], dtype=np.float32))
    nv = np.ascontiguousarray(
        np.asarray(inputs["vis_feats_proj_narr"], dtype=np.float32)
    )
    in_maps = []
    for c in range(NCORES):
        sl = slice(c * RPC, (c + 1) * RPC)
        in_maps.append(
            {
                "vt": np.ascontiguousarray(vp[sl].T),
                "gvt": np.ascontiguousarray(gv[sl].T),
                "nvt": np.ascontiguousarray(nv[sl].T),
                "tpt": np.ascontiguousarray(tp[sl].T),
                "cet": np.ascontiguousarray(ce[sl].T),
                "cer": np.ascontiguousarray(np.roll(ce, -c * RPC, axis=0).T),
            }
        )
    return in_maps


def _run(in_maps, **kwargs):
    from concourse.bass_utils import run_bass_kernel_spmd

    return run_bass_kernel_spmd(
        _get_nc(), in_maps, core_ids=list(range(NCORES)), **kwargs
    )


def _unpack(results, name):
    # [128, MB] per core, local row = m*128 + p -> concat to [BB]
    return np.concatenate([r[name].T.reshape(-1) for r in results])


def _combine(results):
    q = {nm: _unpack(results, nm) for nm in Q_NAMES}

    def cos(dot, ssa, ssb):
        na = np.maximum(np.sqrt(ssa), EPS)
        nb = np.maximum(np.sqrt(ssb), EPS)
        return (dot / (na * nb)).astype(np.float32)

    cos_v = cos(q["q_dot_vf"], q["q_ss_v"], q["q_ss_gv"])
    cos_t = cos(q["q_dot_tc"], q["q_ss_tp"], q["q_ss_ce"])
    vis_loss = np.float32(np.mean((np.float32(1.0) - cos_v)))
    text_loss = np.float32(np.mean((np.float32(1.0) - cos_t)))
    loss = np.float32(vis_loss + text_loss)

    accs = {}
    for x in X_NAMES:
        dp = _unpack(results, "dplus_" + x)
        mx = _unpack(results, "mx_" + x)
        cnt = _unpack(results, "cnt_" + x)
        n_greater = (NTILE * N_ACT + cnt) / 2.0
        ok = (dp >= mx) & (n_greater < 0.5)
        accs[x] = np.float32(100.0 * np.float32(ok.sum()) / np.float32(BB))

    return np.stack(
        [
            vis_loss,
            text_loss,
            loss,
            accs["gv"],
            accs["v"],
            accs["gv"],
            accs["v"],
            accs["nv"],
            accs["nv"],
        ]
    ).astype(np.float32)


def kernel(**inputs):
    return _combine(_run(_make_in_maps(inputs)).results)


# revision 13
# speedup vs baseline: 1.0063x; 1.0063x over previous
"""Trainium2 Bass kernel for nn_EstLossSepEmb (contrastive eval loss_fn).

Strategy (data-parallel over the batch dim, 8 cores, 1024 rows each):
  - Host prep (layout only): slice + transpose each core's query-side tensors
    to [256, 1024]; ship the full caption_emb transposed and *rolled* so each
    core's own 1024 text rows come first (gives compile-time diagonal offsets
    in an SPMD kernel).
  - Device:
      * t-side column scales s = rsqrt(colsum(ce^2)): square (GPSIMD) ->
        column-sums via ones-matmul (PE, fp32r) -> exp(-0.5*ln(ss)) (ACT) ->
        partition broadcast (GPSIMD) -> scaled bf16 copy of ce (GPSIMD).
      * query side (v / gt_v / narr_v) is NOT normalized: per-row argmax is
        scale-invariant in the row; cast to bf16.
      * 3 big sim matmuls [1024,256]x[256,8192] in bf16 (1 cycle/row; bf16
        input rounding perturbs sims ~1e-4 << decision margins ~3e-2).
      * diagonal d_i = <x_i, t_i> from the same bf16 operands (elementwise
        product on GPSIMD + fp32r ones-matmul) so diag-vs-d is consistent
        to ~1e-5; TOL absorbs it.
      * per row decide "argmax == i" as (d+TOL >= rowmax) over the first
        CSPLIT columns of each 2048-column PSUM tile (DVE reduce_max) AND
        count(sim > d+TOL)==0 over the rest (ACT Sign + accumulate).
      * rowwise-cos loss ingredients (dots + sumsqs) via fp32r ones-matmuls
        on the raw fp32 data.
  - Host combine: means / cos / counts -> the 9-vector output.
"""

import os

import numpy as np

BB = 8192
DIM = 256
NCORES = 8
RPC = BB // NCORES  # rows per core = 1024
MB = RPC // 128  # m blocks per core = 8
NTILE = 512
NCH = BB // NTILE  # 16 norm chunks
GRP = 2048  # columns per big PSUM tile (4 banks)
NG = BB // GRP  # 4 groups
KCH = DIM // 128  # 2 k-chunks
TOL = 1e-4
EPS = 1e-8
CSPLIT = int(os.environ.get("K_CSPLIT", "1536"))  # DVE-max cols per big tile
ACT_TOTAL = (GRP - CSPLIT) * NG  # ACT-counted cols per row

Q_NAMES = ["q_dot_vf", "q_ss_v", "q_ss_gv", "q_dot_tc", "q_ss_tp", "q_ss_ce"]
X_NAMES = ["v", "gv", "nv"]

_built = None


def _build_nc():
    import concourse.bacc as bacc
    import concourse.mybir as mybir
    import concourse.tile as tile

    F32 = mybir.dt.float32
    F32R = mybir.dt.float32r
    BF16 = mybir.dt.bfloat16
    AF = mybir.ActivationFunctionType
    AX = mybir.AxisListType

    nc = bacc.Bacc("TRN2", target_bir_lowering=False, debug=False)

    d_in = {}
    for nm in ["vt", "gvt", "nvt", "tpt", "cet"]:
        d_in[nm] = nc.dram_tensor(nm, [DIM, RPC], F32, kind="ExternalInput")
    d_in["cer"] = nc.dram_tensor("cer", [DIM, BB], F32, kind="ExternalInput")

    d_out = {}
    for nm in Q_NAMES:
        d_out[nm] = nc.dram_tensor(nm, [128, MB], F32, kind="ExternalOutput")
    for x in X_NAMES:
        for pre in ["dplus_", "mx_", "cnt_"]:
            nm = pre + x
            d_out[nm] = nc.dram_tensor(nm, [128, MB], F32, kind="ExternalOutput")
    d_out["sdbg"] = nc.dram_tensor("sdbg", [1, BB], F32, kind="ExternalOutput")

    with tile.TileContext(nc) as tc:
        with (
            tc.tile_pool(name="per", bufs=1) as per,
            tc.tile_pool(name="sc", bufs=3) as sc,
            tc.tile_pool(name="pr", bufs=2) as pr,
        ):
            # ---- persistent SBUF tiles ----
            ce_bf = [per.tile([128, BB], BF16, name=f"cebf{k}") for k in range(KCH)]
            xin = {}
            for nm in ["vt", "gvt", "nvt", "tpt", "cet"]:
                xin[nm] = [
                    per.tile([128, RPC], F32, name=f"{nm}{k}") for k in range(KCH)
                ]
            xb = {}
            for nm in ["vt", "gvt", "nvt"]:
                xb[nm] = [
                    per.tile([128, RPC], BF16, name=f"b{nm}{k}") for k in range(KCH)
                ]
            s = per.tile([1, BB], F32, name="s")
            # N=2 ones: fp32 matmul outputs must be 8-byte (2-elem) PSUM lines
            ones_f = per.tile([128, 2], F32, name="ones_f")
            nc.gpsimd.memset(ones_f[:], 1.0)
            ones = per.tile([128, 2], F32R, name="ones")
            nc.vector.tensor_copy(ones[:], ones_f[:])

            negd = {}
            dplus = {}
            mxsl = {}
            cntsl = {}
            mxr = {}
            cntr = {}
            for x in X_NAMES:
                negd[x] = per.tile([128, MB], F32, name=f"negd_{x}")
                dplus[x] = per.tile([128, MB], F32, name=f"dplus_{x}")
                mxsl[x] = per.tile([128, MB, NG], F32, name=f"mxsl_{x}")
                cntsl[x] = per.tile([128, MB, NG], F32, name=f"cntsl_{x}")
                mxr[x] = per.tile([128, MB], F32, name=f"mxr_{x}")
                cntr[x] = per.tile([128, MB], F32, name=f"cntr_{x}")

            # ---- input DMAs + bf16 casts of the query side ----
            for nm in ["vt", "gvt", "nvt", "tpt", "cet"]:
                for k in range(KCH):
                    nc.sync.dma_start(
                        xin[nm][k][:], d_in[nm][k * 128 : (k + 1) * 128, :]
                    )
            for nm in ["vt", "gvt", "nvt"]:
                for k in range(KCH):
                    nc.vector.tensor_copy(xb[nm][k][:], xin[nm][k][:])

            with tc.tile_pool(name="pss", bufs=2, space="PSUM") as pss:
                # ---- t-side scales s=rsqrt(colsum(ce^2)); scaled bf16 ce ----
                for n in range(NCH):
                    cs = slice(n * NTILE, (n + 1) * NTILE)
                    psn = pss.tile([1, NTILE], F32, name="psn")
                    raws = []
                    for k in range(KCH):
                        rawc = sc.tile([128, NTILE], F32, name=f"rawc{k}")
                        nc.sync.dma_start(
                            rawc[:], d_in["cer"][k * 128 : (k + 1) * 128, cs]
                        )
                        raws.append(rawc)
                        sq = sc.tile([128, NTILE], F32R, name=f"sq{k}")
                        nc.gpsimd.tensor_mul(sq[:], rawc[:], rawc[:])
                        nc.tensor.matmul(
                            psn[:],
                            ones[:, 0:1],
                            sq[:],
                            start=(k == 0),
                            stop=(k == KCH - 1),
                        )
                    nc.scalar.activation(s[0:1, cs], psn[:], AF.Ln)
                    nc.scalar.activation(s[0:1, cs], s[0:1, cs], AF.Exp, scale=-0.5)
                    bc = sc.tile([128, NTILE], F32, name="bc")
                    nc.gpsimd.partition_broadcast(bc[:], s[0:1, cs])
                    for k in range(KCH):
                        nc.gpsimd.tensor_mul(ce_bf[k][:, cs], raws[k][:], bc[:])

                nc.sync.dma_start(d_out["sdbg"][:], s[:])

                # ---- diagonal d_i = <x_i, gt_t_i> (+TOL) from bf16 data ----
                for x, nm in zip(X_NAMES, ["vt", "gvt", "nvt"]):
                    prods = []
                    for k in range(KCH):
                        prod = pr.tile([128, RPC], F32R, name=f"prod{k}")
                        nc.gpsimd.tensor_mul(
                            prod[:], xb[nm][k][:], ce_bf[k][:, 0:RPC]
                        )
                        prods.append(prod)
                    psq = pss.tile([128, MB, 2], F32, name="psq")
                    for m in range(MB):
                        ms = slice(m * 128, (m + 1) * 128)
                        for k in range(KCH):
                            nc.tensor.matmul(
                                psq[:, m, :],
                                prods[k][:, ms],
                                ones[:],
                                start=(k == 0),
                                stop=(k == KCH - 1),
                            )
                    nc.vector.tensor_scalar_add(dplus[x][:], psq[:, :, 0:1], TOL)
                    nc.vector.tensor_scalar_mul(negd[x][:], dplus[x][:], -1.0)
                    nc.sync.dma_start(d_out["dplus_" + x][:], dplus[x][:])

                # ---- rowwise-cos loss ingredients (raw fp32) ----
                def colsum_out(name, make_elem):
                    psq = pss.tile([128, MB, 2], F32, name="psq")
                    elems = [make_elem(k) for k in range(KCH)]
                    for m in range(MB):
                        ms = slice(m * 128, (m + 1) * 128)
                        for k in range(KCH):
                            nc.tensor.matmul(
                                psq[:, m, :],
                                elems[k][:, ms],
                                ones[:],
                                start=(k == 0),
                                stop=(k == KCH - 1),
                            )
                    qsb = sc.tile([128, MB], F32, name="qsb")
                    nc.scalar.copy(qsb[:], psq[:, :, 0:1])
                    nc.sync.dma_start(d_out[name][:], qsb[:])

                def mk_mul(a, b):
                    def f(k):
                        prod = pr.tile([128, RPC], F32R, name=f"prod{k}")
                        nc.gpsimd.tensor_mul(prod[:], xin[a][k][:], xin[b][k][:])
                        return prod

                    return f

                def mk_sq(a):
                    def f(k):
                        prod = pr.tile([128, RPC], F32R, name=f"prod{k}")
                        nc.scalar.activation(prod[:], xin[a][k][:], AF.Square)
                        return prod

                    return f

                colsum_out("q_dot_vf", mk_mul("vt", "gvt"))
                colsum_out("q_ss_v", mk_sq("vt"))
                colsum_out("q_ss_gv", mk_sq("gvt"))
                colsum_out("q_dot_tc", mk_mul("tpt", "cet"))
                colsum_out("q_ss_tp", mk_sq("tpt"))
                colsum_out("q_ss_ce", mk_sq("cet"))

            # ---- big sim matmuls + row max / count consumers ----
            with tc.tile_pool(name="psb", bufs=2, space="PSUM") as psb:
                for x in X_NAMES:
                    xt = xb[{"v": "vt", "gv": "gvt", "nv": "nvt"}[x]]
                    for m in range(MB):
                        ms = slice(m * 128, (m + 1) * 128)
                        for g in range(NG):
                            pb = psb.tile([128, GRP], F32, name="pb")
                            for k in range(KCH):
                                for h in range(GRP // NTILE):
                                    c0 = g * GRP + h * NTILE
                                    nc.tensor.matmul(
                                        pb[:, h * NTILE : (h + 1) * NTILE],
                                        xt[k][:, ms],
                                        ce_bf[k][:, c0 : c0 + NTILE],
                                        start=(k == 0),
                                        stop=(k == KCH - 1),
                                    )
                            nc.vector.reduce_max(
                                mxsl[x][:, m, g : g + 1], pb[:, 0:CSPLIT], axis=AX.X
                            )
                            dump = sc.tile([128, GRP - CSPLIT], BF16, name="dump")
                            nc.scalar.activation(
                                dump[:],
                                pb[:, CSPLIT:GRP],
                                AF.Sign,
                                bias=negd[x][:, m : m + 1],
                                accum_out=cntsl[x][:, m, g : g + 1],
                            )

                # ---- fold slots, write outputs ----
                for x in X_NAMES:
                    nc.vector.reduce_max(mxr[x][:], mxsl[x][:], axis=AX.X)
                    nc.vector.reduce_sum(cntr[x][:], cntsl[x][:], axis=AX.X)
                    nc.sync.dma_start(d_out["mx_" + x][:], mxr[x][:])
                    nc.sync.dma_start(d_out["cnt_" + x][:], cntr[x][:])

    nc.compile()
    return nc


def _get_nc():
    global _built
    if _built is None:
        _built = _build_nc()
    return _built


def _make_in_maps(inputs):
    vp = np.ascontiguousarray(np.asarray(inputs["vis_pred"], dtype=np.float32))
    tp = np.ascontiguousarray(np.asarray(inputs["text_pred"], dtype=np.float32))
    gv = np.ascontiguousarray(np.asarray(inputs["vis_feats_proj"], dtype=np.float32))
    ce = np.ascontiguousarray(np.asarray(inputs["caption_emb"], dtype=np.float32))
    nv = np.ascontiguousarray(
        np.asarray(inputs["vis_feats_proj_narr"], dtype=np.float32)
    )
    in_maps = []
    for c in range(NCORES):
        sl = slice(c * RPC, (c + 1) * RPC)
        in_maps.append(
            {
                "vt": np.ascontiguousarray(vp[sl].T),
                "gvt": np.ascontiguousarray(gv[sl].T),
                "nvt": np.ascontiguousarray(nv[sl].T),
                "tpt": np.ascontiguousarray(tp[sl].T),
                "cet": np.ascontiguousarray(ce[sl].T),
                "cer": np.ascontiguousarray(np.roll(ce, -c * RPC, axis=0).T),
            }
        )
    return in_maps


def _run(in_maps, **kwargs):
    from concourse.bass_utils import run_bass_kernel_spmd

    return run_bass_kernel_spmd(
        _get_nc(), in_maps, core_ids=list(range(NCORES)), **kwargs
    )


def _unpack(results, name):
    # [128, MB] per core, local row = m*128 + p -> concat to [BB]
    return np.concatenate([r[name].T.reshape(-1) for r in results])


def _combine(results):
    q = {nm: _unpack(results, nm) for nm in Q_NAMES}

    def cos(dot, ssa, ssb):
        na = np.maximum(np.sqrt(ssa), EPS)
        nb = np.maximum(np.sqrt(ssb), EPS)
        return (dot / (na * nb)).astype(np.float32)

    cos_v = cos(q["q_dot_vf"], q["q_ss_v"], q["q_ss_gv"])
    cos_t = cos(q["q_dot_tc"], q["q_ss_tp"], q["q_ss_ce"])
    vis_loss = np.float32(np.mean((np.float32(1.0) - cos_v)))
    text_loss = np.float32(np.mean((np.float32(1.0) - cos_t)))
    loss = np.float32(vis_loss + text_loss)

    accs = {}
    for x in X_NAMES:
        dp = _unpack(results, "dplus_" + x)
        mx = _unpack(results, "mx_" + x)
        cnt = _unpack(results, "cnt_" + x)
        n_greater = (ACT_TOTAL + cnt) / 2.0
        ok = (dp >= mx) & (n_greater < 0.5)
        accs[x] = np.float32(100.0 * np.float32(ok.sum()) / np.float32(BB))

    return np.stack(
        [
            vis_loss,
            text_loss,
            loss,
            accs["gv"],
            accs["v"],
            accs["gv"],
            accs["v"],
            accs["nv"],
            accs["nv"],
        ]
    ).astype(np.float32)


def kernel(**inputs):
    return _combine(_run(_make_in_maps(inputs)).results)


# revision 16
# speedup vs baseline: 1.5918x; 1.5818x over previous
"""Trainium2 Bass kernel for nn_EstLossSepEmb (contrastive eval loss_fn).

Strategy (data-parallel over the batch dim, 8 cores, 1024 rows each):
  - Host prep (layout only): slice + transpose each core's query-side tensors
    to [256, 1024]; ship the full caption_emb transposed and *rolled* so each
    core's own 1024 text rows come first (gives compile-time diagonal offsets
    in an SPMD kernel).
  - Device:
      * t-side column scales s = rsqrt(colsum(ce^2)): square (DVE) ->
        column-sums via ones-matmul (PE, fp32r) -> batched ACT Ln then
        batched ACT Exp(-0.5*x) (2 table loads total) -> partition broadcast
        (GPSIMD) -> scaled bf16 copy of ce (DVE; ce re-DMAed for this pass).
      * query side (v / gt_v / narr_v) is NOT normalized: per-row argmax is
        scale-invariant in the row; cast to bf16.
      * 3 big sim matmuls [1024,256]x[256,8192] in bf16 (1 cycle/row; bf16
        input rounding perturbs sims ~1e-4 << decision margins ~3e-2).
      * diagonal d_i = <x_i, t_i> from the same bf16 operands (elementwise
        product on GPSIMD + fp32r ones-matmul) so diag-vs-d is consistent
        to ~1e-5; TOL absorbs it.
      * per row decide "argmax == i" as (d+TOL >= rowmax) AND
        count(sim > d+TOL)==0. Each [128,2048] PSUM tile is consumed whole
        either by DVE reduce_max or by ACT Sign+accumulate (per-tile
        assignment balances the two engines; unused slots are memset).
      * rowwise-cos loss ingredients (dots + sumsqs of raw fp32) via M=1
        ones-matmuls after the big phase, output in [1, 1024] row layout.
  - Host combine: means / cos / counts -> the 9-vector output.
"""

import os

import numpy as np

BB = 8192
DIM = 256
NCORES = 8
RPC = BB // NCORES  # rows per core = 1024
MB = RPC // 128  # m blocks per core = 8
NTILE = 512
NCH = BB // NTILE  # 16 norm chunks
GRP = 2048  # columns per big PSUM tile (4 banks)
NG = BB // GRP  # 4 groups
KCH = DIM // 128  # 2 k-chunks
TOL = 1e-4
EPS = 1e-8
NEG_INF = -3.0e38
ACT_MOD = int(os.environ.get("K_ACT_MOD", "3"))  # every ACT_MOD-th tile -> ACT

Q_NAMES = ["q_dot_vf", "q_ss_v", "q_ss_gv", "q_dot_tc", "q_ss_tp", "q_ss_ce"]
X_NAMES = ["v", "gv", "nv"]


def _act_tile(x_i, m, g):
    return ((x_i * MB + m) * NG + g) % ACT_MOD == ACT_MOD - 1


# per-row count baseline: row in m-block m accumulates only (x, m)'s ACT tiles
ACT_COLS_ROW = {
    x: np.tile(
        np.repeat(
            [
                GRP * sum(_act_tile(x_i, m, g) for g in range(NG))
                for m in range(MB)
            ],
            128,
        ),
        NCORES,
    )
    for x_i, x in enumerate(X_NAMES)
}

_built = None


def _build_nc():
    import concourse.bacc as bacc
    import concourse.mybir as mybir
    import concourse.tile as tile

    F32 = mybir.dt.float32
    F32R = mybir.dt.float32r
    BF16 = mybir.dt.bfloat16
    AF = mybir.ActivationFunctionType
    AX = mybir.AxisListType

    nc = bacc.Bacc("TRN2", target_bir_lowering=False, debug=False)

    d_in = {}
    for nm in ["vt", "gvt", "nvt", "tpt", "cet"]:
        d_in[nm] = nc.dram_tensor(nm, [DIM, RPC], F32, kind="ExternalInput")
    d_in["cer"] = nc.dram_tensor("cer", [DIM, BB], F32, kind="ExternalInput")

    d_out = {}
    for nm in Q_NAMES:
        d_out[nm] = nc.dram_tensor(nm, [1, RPC], F32, kind="ExternalOutput")
    for x in X_NAMES:
        for pre in ["dplus_", "mx_", "cnt_"]:
            nm = pre + x
            d_out[nm] = nc.dram_tensor(nm, [128, MB], F32, kind="ExternalOutput")
    d_out["sdbg"] = nc.dram_tensor("sdbg", [1, BB], F32, kind="ExternalOutput")

    with tile.TileContext(nc) as tc:
        with (
            tc.tile_pool(name="per", bufs=1) as per,
            tc.tile_pool(name="sc", bufs=2) as sc,
            tc.tile_pool(name="pr", bufs=2) as pr,
        ):
            # ---- persistent SBUF tiles ----
            ce_bf = [per.tile([128, BB], BF16, name=f"cebf{k}") for k in range(KCH)]
            xin = {}
            for nm in ["vt", "gvt", "nvt", "tpt", "cet"]:
                xin[nm] = [
                    per.tile([128, RPC], F32, name=f"{nm}{k}") for k in range(KCH)
                ]
            xb = {}
            for nm in ["vt", "gvt", "nvt"]:
                xb[nm] = [
                    per.tile([128, RPC], BF16, name=f"b{nm}{k}") for k in range(KCH)
                ]
            s = per.tile([1, BB], F32, name="s")
            # N=2 ones: fp32 matmul outputs must be 8-byte (2-elem) PSUM lines
            ones_f = per.tile([128, 2], F32, name="ones_f")
            nc.gpsimd.memset(ones_f[:], 1.0)
            ones = per.tile([128, 2], F32R, name="ones")
            nc.vector.tensor_copy(ones[:], ones_f[:])

            negd = {}
            dplus = {}
            mxsl = {}
            cntsl = {}
            mxr = {}
            cntr = {}
            for x in X_NAMES:
                negd[x] = per.tile([128, MB], F32, name=f"negd_{x}")
                dplus[x] = per.tile([128, MB], F32, name=f"dplus_{x}")
                mxsl[x] = per.tile([128, MB, NG], F32, name=f"mxsl_{x}")
                cntsl[x] = per.tile([128, MB, NG], F32, name=f"cntsl_{x}")
                mxr[x] = per.tile([128, MB], F32, name=f"mxr_{x}")
                cntr[x] = per.tile([128, MB], F32, name=f"cntr_{x}")
                nc.gpsimd.memset(mxsl[x][:], NEG_INF)
                nc.gpsimd.memset(cntsl[x][:], 0.0)

            # ---- input DMAs + bf16 casts of the query side ----
            for nm in ["vt", "gvt", "nvt", "tpt", "cet"]:
                for k in range(KCH):
                    nc.sync.dma_start(
                        xin[nm][k][:], d_in[nm][k * 128 : (k + 1) * 128, :]
                    )
            for nm in ["vt", "gvt", "nvt"]:
                for k in range(KCH):
                    nc.vector.tensor_copy(xb[nm][k][:], xin[nm][k][:])

            with tc.tile_pool(name="pss", bufs=2, space="PSUM") as pss:
                # ---- t-side scales: ss column sums, batched Ln ----
                for n in range(NCH):
                    cs = slice(n * NTILE, (n + 1) * NTILE)
                    psn = pss.tile([1, NTILE], F32, name="psn")
                    for k in range(KCH):
                        rawc = sc.tile([128, NTILE], F32, name=f"rawc{k}")
                        nc.sync.dma_start(
                            rawc[:], d_in["cer"][k * 128 : (k + 1) * 128, cs]
                        )
                        sq = sc.tile([128, NTILE], F32R, name=f"sq{k}")
                        nc.vector.tensor_mul(sq[:], rawc[:], rawc[:])
                        nc.tensor.matmul(
                            psn[:],
                            ones[:, 0:1],
                            sq[:],
                            start=(k == 0),
                            stop=(k == KCH - 1),
                        )
                    nc.scalar.activation(s[0:1, cs], psn[:], AF.Ln)
                # ---- batched Exp: s = exp(-0.5 * ln(ss)) ----
                for n in range(NCH):
                    cs = slice(n * NTILE, (n + 1) * NTILE)
                    nc.scalar.activation(s[0:1, cs], s[0:1, cs], AF.Exp, scale=-0.5)
                # ---- scaled bf16 ce (fresh DMA of the raw data) ----
                for n in range(NCH):
                    cs = slice(n * NTILE, (n + 1) * NTILE)
                    bc = sc.tile([128, NTILE], F32, name="bc")
                    nc.gpsimd.partition_broadcast(bc[:], s[0:1, cs])
                    for k in range(KCH):
                        rawc2 = sc.tile([128, NTILE], F32, name=f"rawc2{k}")
                        nc.sync.dma_start(
                            rawc2[:], d_in["cer"][k * 128 : (k + 1) * 128, cs]
                        )
                        nc.vector.tensor_mul(ce_bf[k][:, cs], rawc2[:], bc[:])

                nc.sync.dma_start(d_out["sdbg"][:], s[:])

                # ---- diagonal d_i = <x_i, gt_t_i> (+TOL) from bf16 data ----
                for x, nm in zip(X_NAMES, ["vt", "gvt", "nvt"]):
                    prods = []
                    for k in range(KCH):
                        prod = pr.tile([128, RPC], F32R, name=f"prod{k}")
                        nc.gpsimd.tensor_mul(
                            prod[:], xb[nm][k][:], ce_bf[k][:, 0:RPC]
                        )
                        prods.append(prod)
                    psq = pss.tile([128, MB, 2], F32, name="psq")
                    for m in range(MB):
                        ms = slice(m * 128, (m + 1) * 128)
                        for k in range(KCH):
                            nc.tensor.matmul(
                                psq[:, m, :],
                                prods[k][:, ms],
                                ones[:],
                                start=(k == 0),
                                stop=(k == KCH - 1),
                            )
                    nc.vector.tensor_scalar_add(dplus[x][:], psq[:, :, 0:1], TOL)
                    nc.vector.tensor_scalar_mul(negd[x][:], dplus[x][:], -1.0)
                    nc.sync.dma_start(d_out["dplus_" + x][:], dplus[x][:])

            # ---- big sim matmuls + row max / count consumers ----
            with tc.tile_pool(name="psb", bufs=2, space="PSUM") as psb:
                for x_i, x in enumerate(X_NAMES):
                    xt = xb[{"v": "vt", "gv": "gvt", "nv": "nvt"}[x]]
                    for m in range(MB):
                        ms = slice(m * 128, (m + 1) * 128)
                        for g in range(NG):
                            pb = psb.tile([128, GRP], F32, name="pb")
                            for k in range(KCH):
                                for h in range(GRP // NTILE):
                                    c0 = g * GRP + h * NTILE
                                    nc.tensor.matmul(
                                        pb[:, h * NTILE : (h + 1) * NTILE],
                                        xt[k][:, ms],
                                        ce_bf[k][:, c0 : c0 + NTILE],
                                        start=(k == 0),
                                        stop=(k == KCH - 1),
                                    )
                            if _act_tile(x_i, m, g):
                                dump = sc.tile([128, GRP], BF16, name="dump")
                                nc.scalar.activation(
                                    dump[:],
                                    pb[:],
                                    AF.Sign,
                                    bias=negd[x][:, m : m + 1],
                                    accum_out=cntsl[x][:, m, g : g + 1],
                                )
                            else:
                                nc.vector.reduce_max(
                                    mxsl[x][:, m, g : g + 1], pb[:], axis=AX.X
                                )

            # ---- loss colsums (M=1 ones-matmuls, [1, RPC] outputs) ----
            with tc.tile_pool(name="pst", bufs=2, space="PSUM") as pst:

                def colsum_out(name, make_elem):
                    elems = [make_elem(k) for k in range(KCH)]
                    qsb = sc.tile([1, RPC], F32, name="qsb")
                    for half in range(RPC // NTILE):
                        hs = slice(half * NTILE, (half + 1) * NTILE)
                        psl = pst.tile([1, NTILE], F32, name="psl")
                        for k in range(KCH):
                            nc.tensor.matmul(
                                psl[:],
                                ones[:, 0:1],
                                elems[k][:, hs],
                                start=(k == 0),
                                stop=(k == KCH - 1),
                            )
                        nc.scalar.copy(qsb[0:1, hs], psl[:])
                    nc.sync.dma_start(d_out[name][:], qsb[:])

                def mk_mul(a, b):
                    def f(k):
                        prod = pr.tile([128, RPC], F32R, name=f"prod{k}")
                        nc.gpsimd.tensor_mul(prod[:], xin[a][k][:], xin[b][k][:])
                        return prod

                    return f

                colsum_out("q_dot_vf", mk_mul("vt", "gvt"))
                colsum_out("q_ss_v", mk_mul("vt", "vt"))
                colsum_out("q_ss_gv", mk_mul("gvt", "gvt"))
                colsum_out("q_dot_tc", mk_mul("tpt", "cet"))
                colsum_out("q_ss_tp", mk_mul("tpt", "tpt"))
                colsum_out("q_ss_ce", mk_mul("cet", "cet"))

                # ---- fold slots, write outputs ----
                for x in X_NAMES:
                    nc.vector.reduce_max(mxr[x][:], mxsl[x][:], axis=AX.X)
                    nc.vector.reduce_sum(cntr[x][:], cntsl[x][:], axis=AX.X)
                    nc.sync.dma_start(d_out["mx_" + x][:], mxr[x][:])
                    nc.sync.dma_start(d_out["cnt_" + x][:], cntr[x][:])

    nc.compile()
    return nc


def _get_nc():
    global _built
    if _built is None:
        _built = _build_nc()
    return _built


def _make_in_maps(inputs):
    vp = np.ascontiguousarray(np.asarray(inputs["vis_pred"], dtype=np.float32))
    tp = np.ascontiguousarray(np.asarray(inputs["text_pred"], dtype=np.float32))
    gv = np.ascontiguousarray(np.asarray(inputs["vis_feats_proj"], dtype=np.float32))
    ce = np.ascontiguousarray(np.asarray(inputs["caption_emb"], dtype=np.float32))
    nv = np.ascontiguousarray(
        np.asarray(inputs["vis_feats_proj_narr"], dtype=np.float32)
    )
    in_maps = []
    for c in range(NCORES):
        sl = slice(c * RPC, (c + 1) * RPC)
        in_maps.append(
            {
                "vt": np.ascontiguousarray(vp[sl].T),
                "gvt": np.ascontiguousarray(gv[sl].T),
                "nvt": np.ascontiguousarray(nv[sl].T),
                "tpt": np.ascontiguousarray(tp[sl].T),
                "cet": np.ascontiguousarray(ce[sl].T),
                "cer": np.ascontiguousarray(np.roll(ce, -c * RPC, axis=0).T),
            }
        )
    return in_maps


def _run(in_maps, **kwargs):
    from concourse.bass_utils import run_bass_kernel_spmd

    return run_bass_kernel_spmd(
        _get_nc(), in_maps, core_ids=list(range(NCORES)), **kwargs
    )


def _unpack(results, name):
    # [128, MB] per core, local row = m*128 + p -> concat to [BB]
    return np.concatenate([r[name].T.reshape(-1) for r in results])


def _unpack_row(results, name):
    # [1, RPC] per core -> concat to [BB]
    return np.concatenate([r[name][0] for r in results])


def _combine(results):
    q = {nm: _unpack_row(results, nm) for nm in Q_NAMES}

    def cos(dot, ssa, ssb):
        na = np.maximum(np.sqrt(ssa), EPS)
        nb = np.maximum(np.sqrt(ssb), EPS)
        return (dot / (na * nb)).astype(np.float32)

    cos_v = cos(q["q_dot_vf"], q["q_ss_v"], q["q_ss_gv"])
    cos_t = cos(q["q_dot_tc"], q["q_ss_tp"], q["q_ss_ce"])
    vis_loss = np.float32(np.mean((np.float32(1.0) - cos_v)))
    text_loss = np.float32(np.mean((np.float32(1.0) - cos_t)))
    loss = np.float32(vis_loss + text_loss)

    accs = {}
    for x in X_NAMES:
        dp = _unpack(results, "dplus_" + x)
        mx = _unpack(results, "mx_" + x)
        cnt = _unpack(results, "cnt_" + x)
        n_greater = (ACT_COLS_ROW[x] + cnt) / 2.0
        ok = (dp >= mx) & (n_greater < 0.5)
        accs[x] = np.float32(100.0 * np.float32(ok.sum()) / np.float32(BB))

    return np.stack(
        [
            vis_loss,
            text_loss,
            loss,
            accs["gv"],
            accs["v"],
            accs["gv"],
            accs["v"],
            accs["nv"],
            accs["nv"],
        ]
    ).astype(np.float32)


def kernel(**inputs):
    return _combine(_run(_make_in_maps(inputs)).results)
